# revision 2
# baseline (speedup 1.0000x reference)
"""Integrate-and-fire scan (T=8) on Trainium2, data-parallel over 8 NeuronCores.

Reference semantics per element, scanned over t:
    mem = mem + x[t]; spike = (mem - 1 > 0); mem = mem - spike

Sharding: batch dim (axis 1 of x / axis 0 of mem0) split 4-per-core across 8
cores; the scan is elementwise so no cross-core communication is needed.

Formulation (prefix-sum): with S_t = mem0 + sum_{s<=t} x_s (fp32 running sum,
mem0 folded into x[0] on the host) and N_t = floor(S_t) (spike count through
t), the spike train is spike_t = N_t - N_{t-1}.  Engine mapping per core:

  - DVE:  S_t = S_{t-1} + x_t            (7 tensor_add passes — its only work)
  - ACT:  N_t = int32(S_t - 0.5)          (f32->i32 convert rounds to nearest,
                                           so this is floor(S_t) exactly away
                                           from integer-valued S_t)
  - POOL: Nb_t = bf16(N_t)                (exact: N_t <= 8)
  - PE:   pack += c_t * Nb_t in PSUM with c = [-1,-2,...,-64,+128]; the
          telescoped sum equals sum_t 2^t spike_t in [0,255], an exact f32 int
  - ACT:  drain PSUM -> u8 out tile; host unpacks the 8 bits per element

Output DMA is 8x smaller than raw u8 spikes (0.6 MB vs 4.8 MB per core), so
device traffic is x in (19.3 MB) + packed spikes out (0.6 MB) per core and the
kernel runs at the HBM read roofline (~358 GB/s per core).
"""

import os
import sys

if "/opt/trn_rl_repo" not in sys.path:
    sys.path.insert(0, "/opt/trn_rl_repo")

import numpy as np
import ml_dtypes

import concourse.bass as bass  # noqa: F401  (registers engine classes)
import concourse.tile as tile
from concourse import bacc, mybir
from concourse.bass_utils import run_bass_kernel_spmd

T, B, C, H, W = 8, 32, 3, 224, 224
NCORES = 8
BPC = B // NCORES            # 4 batch elements per core
E = BPC * C * H * W          # 602112 elements per (core, timestep)
P = 128
F = E // P                   # 4704 free-dim columns
F32 = mybir.dt.float32
BF16 = mybir.dt.bfloat16
I32 = mybir.dt.int32
U8 = mybir.dt.uint8

# Tunables (env-overridable for A/B testing)
WB = int(os.environ.get("IAF_WB", "1176"))      # block width (divides 4704)
NBLK = F // WB
MMW = 512                                        # matmul free-dim per PSUM bank
# Cast engine for int32->bf16: "pool" | "dve" | "mix:<k>" (every k-th on DVE)
CAST = os.environ.get("IAF_CAST", "pool")
# Drain engine: "act" | "dve"
DRAIN = os.environ.get("IAF_DRAIN", "act")
X_BUFS = int(os.environ.get("IAF_X_BUFS", "10"))
S_BUFS = int(os.environ.get("IAF_S_BUFS", "3"))
N_BUFS = int(os.environ.get("IAF_N_BUFS", "3"))
NB_BUFS = int(os.environ.get("IAF_NB_BUFS", "4"))
O_BUFS = int(os.environ.get("IAF_O_BUFS", "2"))
PS_BUFS = int(os.environ.get("IAF_PS_BUFS", "2"))
OUT_ENG = os.environ.get("IAF_OUTQ", "gpsimd")   # out-DMA engine

_compiled_nc = None

# PE pack weights: pack = sum_t c_t * N_t = sum_t 2^t spike_t
PACK_C = [-float(2 ** t) for t in range(T - 1)] + [float(2 ** (T - 1))]


def _build():
    nc = bacc.Bacc("TRN2", target_bir_lowering=False, debug=False,
                   num_devices=NCORES)
    x = nc.dram_tensor("x", [T, P, F], F32, kind="ExternalInput").ap()
    wts = nc.dram_tensor("wts", [T * P, P], BF16, kind="ExternalInput").ap()
    out = nc.dram_tensor("out", [P, F], U8, kind="ExternalOutput").ap()

    with tile.TileContext(nc) as tc:
        with tc.tile_pool(name="const", bufs=1) as c_pool, \
             tc.tile_pool(name="xin", bufs=X_BUFS) as x_pool, \
             tc.tile_pool(name="s", bufs=S_BUFS) as s_pool, \
             tc.tile_pool(name="n", bufs=N_BUFS) as n_pool, \
             tc.tile_pool(name="nb", bufs=NB_BUFS) as nb_pool, \
             tc.tile_pool(name="o", bufs=O_BUFS) as o_pool, \
             tc.tile_pool(name="ps", bufs=PS_BUFS, space="PSUM") as ps_pool:

            bneg = c_pool.tile([P, 1], F32)
            nc.vector.memset(bneg[:], -0.5)
            wt_tiles = []
            for t in range(T):
                wt = c_pool.tile([P, P], BF16, tag=f"wt{t}")
                nc.sync.dma_start(out=wt[:], in_=wts[t * P:(t + 1) * P, :])
                wt_tiles.append(wt)

            out_eng = {"gpsimd": nc.gpsimd, "sync": nc.sync,
                       "scalar": nc.scalar}[OUT_ENG]

            cast_idx = 0
            for b in range(NBLK):
                col0 = b * WB
                psum = ps_pool.tile([P, WB], F32)
                s_prev = None
                for t in range(T):
                    xt = x_pool.tile([P, WB], F32)
                    nc.sync.dma_start(out=xt[:],
                                      in_=x[t, :, col0:col0 + WB])
                    if t == 0:
                        st = xt
                    else:
                        st = s_pool.tile([P, WB], F32)
                        nc.vector.tensor_add(st[:], s_prev[:], xt[:])
                    s_prev = st
                    nt = n_pool.tile([P, WB], I32)
                    nc.scalar.activation(nt[:], st[:],
                                         mybir.ActivationFunctionType.Identity,
                                         bias=bneg[:], scale=1.0)
                    nbt = nb_pool.tile([P, WB], BF16)
                    if CAST == "pool":
                        on_dve = False
                    elif CAST == "dve":
                        on_dve = True
                    else:
                        k = int(CAST.split(":")[1])
                        on_dve = (cast_idx % k) == (k - 1)
                    cast_idx += 1
                    if on_dve:
                        nc.vector.tensor_copy(nbt[:], nt[:])
                    else:
                        nc.gpsimd.tensor_copy(nbt[:], nt[:])
                    for m0 in range(0, WB, MMW):
                        m1 = min(m0 + MMW, WB)
                        nc.tensor.matmul(psum[:, m0:m1], wt_tiles[t][:],
                                         nbt[:, m0:m1],
                                         start=(t == 0), stop=(t == T - 1))
                ot = o_pool.tile([P, WB], U8)
                if DRAIN == "act":
                    nc.scalar.activation(ot[:], psum[:],
                                         mybir.ActivationFunctionType.Identity)
                else:
                    nc.vector.tensor_copy(ot[:], psum[:])
                out_eng.dma_start(out=out[:, col0:col0 + WB], in_=ot[:])
    nc.compile()
    return nc


def _get_nc():
    global _compiled_nc
    if _compiled_nc is None:
        _compiled_nc = _build()
    return _compiled_nc


def _make_wts():
    ident = np.eye(P, dtype=np.float32)
    w = np.concatenate([c * ident for c in PACK_C], axis=0)
    return w.astype(ml_dtypes.bfloat16)


def _run(x, mem0, trace=False):
    nc = _get_nc()
    wts = _make_wts()
    in_maps = []
    for i in range(NCORES):
        bsl = slice(i * BPC, (i + 1) * BPC)
        xi = np.ascontiguousarray(x[:, bsl]).reshape(T, P, F)
        # Fold the initial membrane into the first timestep (bit-exact fp32
        # add, same rounding the device add would produce).
        xi[0] += mem0[bsl].reshape(P, F)
        in_maps.append({"x": xi, "wts": wts})
    res = run_bass_kernel_spmd(nc, in_maps, list(range(NCORES)), trace=trace)
    full = np.empty((T, B, C, H, W), dtype=np.float32)
    shifts = np.arange(T, dtype=np.uint8)[:, None, None]
    for i in range(NCORES):
        packed = res.results[i]["out"]  # [P, F] u8, bit t = spike_t
        bits = (packed[None, :, :] >> shifts) & np.uint8(1)
        full[:, i * BPC:(i + 1) * BPC] = bits.astype(np.float32).reshape(
            T, BPC, C, H, W)
    return full, res


def kernel(x, mem0):
    x = np.asarray(x, dtype=np.float32)
    mem0 = np.asarray(mem0, dtype=np.float32)
    full, _ = _run(x, mem0, trace=False)
    return full


# revision 4
# speedup vs baseline: 2.0861x; 2.0861x over previous
"""Integrate-and-fire scan (T=8) on Trainium2, data-parallel over 8 NeuronCores.

Reference semantics per element, scanned over t:
    mem = mem + x[t]; spike = (mem - 1 > 0); mem = mem - spike

Sharding: batch dim (axis 1 of x / axis 0 of mem0) split 4-per-core across 8
cores; the scan is elementwise so no cross-core communication is needed.

Formulation (prefix-sum): with S_t = mem0 + sum_{s<=t} x_s (fp32 running sum,
mem0 folded into x[0] on the host) and N_t = floor(S_t) (spike count through
t), the spike train is spike_t = N_t - N_{t-1}.  Engine mapping per core:

  - DVE:  S_t = S_{t-1} + x_t            (7 tensor_add passes — its only work)
  - ACT:  N_t = int32(S_t - 0.5)          (f32->i32 convert rounds to nearest,
                                           so this is floor(S_t) exactly away
                                           from integer-valued S_t)
  - POOL: Nb_t = bf16(N_t)                (exact: N_t <= 8)
  - PE:   pack += c_t * Nb_t in PSUM with c = [-1,-2,...,-64,+128]; the
          telescoped sum equals sum_t 2^t spike_t in [0,255], an exact f32 int
  - ACT:  drain PSUM -> u8 out tile; host unpacks the 8 bits per element

Output DMA is 8x smaller than raw u8 spikes (0.6 MB vs 4.8 MB per core), so
device traffic is x in (19.3 MB) + packed spikes out (0.6 MB) per core and the
kernel runs at the HBM read roofline (~358 GB/s per core).
"""

import os
import sys

if "/opt/trn_rl_repo" not in sys.path:
    sys.path.insert(0, "/opt/trn_rl_repo")

import numpy as np
import ml_dtypes

import concourse.bass as bass  # noqa: F401  (registers engine classes)
import concourse.tile as tile
from concourse import bacc, mybir
from concourse.bass_utils import run_bass_kernel_spmd

T, B, C, H, W = 8, 32, 3, 224, 224
NCORES = 8
BPC = B // NCORES            # 4 batch elements per core
E = BPC * C * H * W          # 602112 elements per (core, timestep)
P = 128
F = E // P                   # 4704 free-dim columns
F32 = mybir.dt.float32
BF16 = mybir.dt.bfloat16
I32 = mybir.dt.int32
U8 = mybir.dt.uint8

# Tunables (env-overridable for A/B testing)
WB = int(os.environ.get("IAF_WB", "1568"))      # block width (divides 4704)
NBLK = F // WB
MMW = 512                                        # matmul free-dim per PSUM bank
# Engine pattern strings, cycled over the flat (block, t) unit index:
#   'v' = VectorE (2x-mode single-src), 'a' = ScalarE, 'p' = GpSimd.
# floor: S -> int32(S - 0.5); cast: int32 -> bf16 for the PE pack.
FLOOR_PAT = os.environ.get("IAF_FLOOR_PAT", "vva")
CAST_PAT = os.environ.get("IAF_CAST_PAT", "a")
# Drain engine: "act" | "dve"
DRAIN = os.environ.get("IAF_DRAIN", "act")
X_BUFS = int(os.environ.get("IAF_X_BUFS", "10"))
S_BUFS = int(os.environ.get("IAF_S_BUFS", "3"))
N_BUFS = int(os.environ.get("IAF_N_BUFS", "4"))
NB_BUFS = int(os.environ.get("IAF_NB_BUFS", "6"))
O_BUFS = int(os.environ.get("IAF_O_BUFS", "2"))
PS_BUFS = int(os.environ.get("IAF_PS_BUFS", "2"))
OUT_ENG = os.environ.get("IAF_OUTQ", "gpsimd")   # out-DMA engine

_compiled_nc = None

# PE pack weights: pack = sum_t c_t * N_t = sum_t 2^t spike_t
PACK_C = [-float(2 ** t) for t in range(T - 1)] + [float(2 ** (T - 1))]


def _build():
    nc = bacc.Bacc("TRN2", target_bir_lowering=False, debug=False,
                   num_devices=NCORES)
    x = nc.dram_tensor("x", [T, P, F], F32, kind="ExternalInput").ap()
    wts = nc.dram_tensor("wts", [T * P, P], BF16, kind="ExternalInput").ap()
    out = nc.dram_tensor("out", [P, F], U8, kind="ExternalOutput").ap()

    with tile.TileContext(nc) as tc:
        with tc.tile_pool(name="const", bufs=1) as c_pool, \
             tc.tile_pool(name="xin", bufs=X_BUFS) as x_pool, \
             tc.tile_pool(name="s", bufs=S_BUFS) as s_pool, \
             tc.tile_pool(name="n", bufs=N_BUFS) as n_pool, \
             tc.tile_pool(name="nb", bufs=NB_BUFS) as nb_pool, \
             tc.tile_pool(name="o", bufs=O_BUFS) as o_pool, \
             tc.tile_pool(name="ps", bufs=PS_BUFS, space="PSUM") as ps_pool:

            bneg = c_pool.tile([P, 1], F32)
            nc.vector.memset(bneg[:], -0.5)
            wt_tiles = []
            for t in range(T):
                wt = c_pool.tile([P, P], BF16, tag=f"wt{t}")
                nc.sync.dma_start(out=wt[:], in_=wts[t * P:(t + 1) * P, :])
                wt_tiles.append(wt)

            out_eng = {"gpsimd": nc.gpsimd, "sync": nc.sync,
                       "scalar": nc.scalar}[OUT_ENG]

            unit = 0
            for b in range(NBLK):
                col0 = b * WB
                psum = ps_pool.tile([P, WB], F32)
                s_prev = None
                for t in range(T):
                    xt = x_pool.tile([P, WB], F32)
                    nc.sync.dma_start(out=xt[:],
                                      in_=x[t, :, col0:col0 + WB])
                    if t == 0:
                        st = xt
                    else:
                        st = s_pool.tile([P, WB], F32)
                        nc.vector.tensor_add(st[:], s_prev[:], xt[:])
                    s_prev = st
                    floor_eng = FLOOR_PAT[unit % len(FLOOR_PAT)]
                    cast_eng = CAST_PAT[unit % len(CAST_PAT)]
                    unit += 1
                    nt = n_pool.tile([P, WB], I32)
                    if floor_eng == "v":
                        # int32 output dtype converts with round-to-nearest,
                        # same mechanism as the ACT path
                        nc.vector.tensor_scalar(
                            out=nt[:], in0=st[:], scalar1=-0.5, scalar2=None,
                            op0=mybir.AluOpType.add)
                    elif floor_eng == "p":
                        nc.gpsimd.tensor_scalar(
                            out=nt[:], in0=st[:], scalar1=-0.5, scalar2=None,
                            op0=mybir.AluOpType.add)
                    else:
                        nc.scalar.activation(
                            nt[:], st[:],
                            mybir.ActivationFunctionType.Identity,
                            bias=bneg[:], scale=1.0)
                    nbt = nb_pool.tile([P, WB], BF16)
                    if cast_eng == "v":
                        nc.vector.tensor_copy(nbt[:], nt[:])
                    elif cast_eng == "p":
                        nc.gpsimd.tensor_copy(nbt[:], nt[:])
                    else:
                        nc.scalar.activation(
                            nbt[:], nt[:],
                            mybir.ActivationFunctionType.Identity)
                    for m0 in range(0, WB, MMW):
                        m1 = min(m0 + MMW, WB)
                        nc.tensor.matmul(psum[:, m0:m1], wt_tiles[t][:],
                                         nbt[:, m0:m1],
                                         start=(t == 0), stop=(t == T - 1))
                ot = o_pool.tile([P, WB], U8)
                if DRAIN == "act":
                    nc.scalar.activation(ot[:], psum[:],
                                         mybir.ActivationFunctionType.Identity)
                else:
                    nc.vector.tensor_copy(ot[:], psum[:])
                out_eng.dma_start(out=out[:, col0:col0 + WB], in_=ot[:])
    nc.compile()
    return nc


def _get_nc():
    global _compiled_nc
    if _compiled_nc is None:
        _compiled_nc = _build()
    return _compiled_nc


def _make_wts():
    ident = np.eye(P, dtype=np.float32)
    w = np.concatenate([c * ident for c in PACK_C], axis=0)
    return w.astype(ml_dtypes.bfloat16)


def _run(x, mem0, trace=False):
    nc = _get_nc()
    wts = _make_wts()
    in_maps = []
    for i in range(NCORES):
        bsl = slice(i * BPC, (i + 1) * BPC)
        xi = np.ascontiguousarray(x[:, bsl]).reshape(T, P, F)
        # Fold the initial membrane into the first timestep (bit-exact fp32
        # add, same rounding the device add would produce).
        xi[0] += mem0[bsl].reshape(P, F)
        in_maps.append({"x": xi, "wts": wts})
    res = run_bass_kernel_spmd(nc, in_maps, list(range(NCORES)), trace=trace)
    full = np.empty((T, B, C, H, W), dtype=np.float32)
    shifts = np.arange(T, dtype=np.uint8)[:, None, None]
    for i in range(NCORES):
        packed = res.results[i]["out"]  # [P, F] u8, bit t = spike_t
        bits = (packed[None, :, :] >> shifts) & np.uint8(1)
        full[:, i * BPC:(i + 1) * BPC] = bits.astype(np.float32).reshape(
            T, BPC, C, H, W)
    return full, res


def kernel(x, mem0):
    x = np.asarray(x, dtype=np.float32)
    mem0 = np.asarray(mem0, dtype=np.float32)
    full, _ = _run(x, mem0, trace=False)
    return full
